# revision 1
# baseline (speedup 1.0000x reference)
"""Trainium2 Bass kernel for nn_Kernel_12281761695451725822_53472342835843.

Computation (per sample n, channel c):
  t3 = Conv1x5(x, w3)                      (channel-mixing 1x5 conv, pad 2)
  t7 = sum over 9 (oh,ow) terms of w7[c,3*ow+oh] * max(A_ohow, B_ohow)
       A = x[h, w+2*oh-2] (zero pad), B = x[h+oh-1, w+2*ow-3] (zero pad; w=0 wraps)
  out = t7 * t3

Strategy (pure data-parallel over batch: 2 samples per core, 8 cores):
  - SBUF layout: 128 partitions = (n_local, c); free dim = padded canvas (h, w).
  - The 9 max terms reduce to 6 shared pair-max tensors M_delta (max is symmetric).
  - DVE computes the 6 maxes in fp16 (2x mode); even alignment via a 1-shifted
    canvas copy so both operands of every max are 4B-aligned.
  - PE (fp16 matmuls, 1 cycle/row) computes t3 (5 block-diag taps) and the 9
    weighted tap accumulations of t7 (diagonal lhsT), accumulating in PSUM.
  - ACT drains both PSUM accumulators to SBUF fp16; DVE does the final t7*t3.
  - w=0 output column is recomputed exactly (roll-by-1 wrap) and patched in.
"""

import numpy as np

N, C, H, W = 16, 64, 128, 128
NCORES = 8
NLOC = N // NCORES          # samples per core
P = 128                     # partitions = NLOC * C
ROWS = H + 2                # canvas rows, storage row = h + 1
COLS = 144                  # canvas cols, storage col = u + UOFF, u in [-9, 135)
UOFF = 9
HS = 8                      # strip height (output rows per strip)
NSTRIPS = H // HS
OUT_ROWS = 16               # output staging rows per DMA
MCOLS = 136                 # M tensor storage stride (cols), valid [0, 134)

# term i: (oh, ow, (dh, dw) of shared max, read offset (dr, du) into M)
TERMS = [
    (0, 0, (1, 1), -1, -3),
    (0, 1, (1, -1), -1, -1),
    (0, 2, (1, -3), -1, 1),
    (1, 0, (0, 3), 0, -3),
    (1, 1, (0, 1), 0, -1),
    (1, 2, (0, 1), 0, 0),
    (2, 0, (1, -5), 0, 2),
    (2, 1, (1, -3), 0, 2),
    (2, 2, (1, -1), 0, 2),
]
DELTAS = [(1, 1), (1, -1), (1, -3), (0, 3), (0, 1), (1, -5)]


def build_host_weights(w3, w7):
    """Host-side packing of the conv weights into PE lhsT layouts (fp16)."""
    w3 = np.asarray(w3, dtype=np.float32)
    w7 = np.asarray(w7, dtype=np.float32)
    wt3 = np.zeros((5, P, P), dtype=np.float16)
    for k in range(5):
        blk = w3[:, :, 0, k].T.astype(np.float16)  # [ci, co]
        for n in range(NLOC):
            wt3[k, n * C:(n + 1) * C, n * C:(n + 1) * C] = blk
    wt7 = np.zeros((9, P, P), dtype=np.float16)
    w7c = np.zeros((P, 9), dtype=np.float16)
    for i, (oh, ow, _d, _dr, _du) in enumerate(TERMS):
        kidx = 3 * ow + oh
        vals = np.concatenate([w7[:, kidx], w7[:, kidx]]).astype(np.float16)  # [P]
        wt7[i, np.arange(P), np.arange(P)] = vals
        w7c[:, i] = vals
    return wt3, wt7, w7c


def build_program():
    """Build and compile the single-core Bass/Tile program (SPMD-replicated)."""
    import concourse.bacc as bacc
    import concourse.tile as tile
    import concourse.mybir as mybir

    fp16 = mybir.dt.float16
    fp32 = mybir.dt.float32
    AOT = mybir.AluOpType

    nc = bacc.Bacc("TRN2", target_bir_lowering=False, debug=False,
                   enable_asserts=False, num_devices=1)
    x_in = nc.dram_tensor("x_in", [P, H, W], fp32, kind="ExternalInput")
    wt3_d = nc.dram_tensor("wt3", [5, P, P], fp16, kind="ExternalInput")
    wt7_d = nc.dram_tensor("wt7", [9, P, P], fp16, kind="ExternalInput")
    w7c_d = nc.dram_tensor("w7c", [P, 9], fp16, kind="ExternalInput")
    out_d = nc.dram_tensor("out", [P, H, W], fp32, kind="ExternalOutput")

    with tile.TileContext(nc) as tc:
        with (
            tc.tile_pool(name="persist", bufs=1) as persist,
            tc.tile_pool(name="mpool", bufs=2) as mpool,
            tc.tile_pool(name="ppool", bufs=2, space="PSUM") as ppool,
            tc.tile_pool(name="spool", bufs=3) as spool,
            tc.tile_pool(name="opool", bufs=2) as opool,
            tc.tile_pool(name="fixp", bufs=2) as fixp,
        ):
            canvas = persist.tile([P, ROWS, COLS], fp16, tag="canvas")
            canvaso = persist.tile([P, ROWS, COLS], fp16, tag="canvaso")
            wt3_s = persist.tile([P, 5, P], fp16, tag="wt3")
            wt7_s = persist.tile([P, 9, P], fp16, tag="wt7")
            w7c_s = persist.tile([P, 9], fp16, tag="w7c")
            t7c0 = persist.tile([P, H], fp16, tag="t7c0")

            # weights in
            nc.sync.dma_start(out=wt3_s, in_=wt3_d.ap().rearrange("k a b -> a k b"))
            nc.sync.dma_start(out=wt7_s, in_=wt7_d.ap().rearrange("k a b -> a k b"))
            nc.sync.dma_start(out=w7c_s, in_=w7c_d.ap())

            # canvas: zero borders, then stream x in 32-row chunks so strip
            # compute overlaps the input pipeline: dense cast-DMA to staging,
            # ACT copy into the padded interior, flat-shift DMA for canvaso.
            stgt = persist.tile([P, H, W], fp16, tag="stg")
            nc.vector.memset(canvas[:, 0, :], 0.0)
            nc.vector.memset(canvas[:, H + 1, :], 0.0)
            nc.vector.memset(canvas[:, 1:H + 1, 0:UOFF], 0.0)
            nc.vector.memset(canvas[:, 1:H + 1, UOFF + W:COLS], 0.0)
            cflat = canvas.rearrange("p r c -> p (r c)")
            coflat = canvaso.rearrange("p r c -> p (r c)")
            CH = 32
            for cb in range(H // CH):
                h0 = cb * CH
                nc.gpsimd.dma_start(out=stgt[:, h0:h0 + CH, :],
                                    in_=x_in.ap()[:, h0:h0 + CH, :])
                nc.scalar.copy(out=canvas[:, 1 + h0:1 + h0 + CH, UOFF:UOFF + W],
                               in_=stgt[:, h0:h0 + CH, :])
                # canvaso rows [h0 .. h0+CH): flat shift; the chunk's last
                # element reads canvas[r_next, 0], a zeroed border column.
                f0 = (1 + h0) * COLS if cb > 0 else 0
                f1 = (1 + h0 + CH) * COLS if cb < H // CH - 1 else ROWS * COLS - 1
                nc.gpsimd.dma_start(out=coflat[:, f0:f1],
                                    in_=cflat[:, f0 + 1:f1 + 1])

            # --- w=0 column fixup values: t7c0[p, h] = sum_i w_i * max(A_i, B_i) at w=0
            for i, (oh, ow, _d, _dr, _du) in enumerate(TERMS):
                tmp = fixp.tile([P, H], fp16, tag="fixtmp")
                a_ap = canvas[:, 1:1 + H, 2 * oh + 7]
                b_ap = canvas[:, oh:oh + H, 134 + 2 * ow]
                nc.vector.tensor_tensor(tmp, a_ap, b_ap, AOT.max)
                nc.vector.scalar_tensor_tensor(
                    out=t7c0, in0=tmp, scalar=w7c_s[:, i:i + 1],
                    in1=(tmp if i == 0 else t7c0),
                    op0=AOT.mult, op1=(AOT.bypass if i == 0 else AOT.add))

            # --- main strip loop
            for s in range(NSTRIPS):
                r0 = s * HS
                # 6 shared max tensors for this strip (rows r0-1 .. r0+HS-1)
                mts = {}
                for di, (dh, dw) in enumerate(DELTAS):
                    mt = mpool.tile([P, HS + 1, MCOLS], fp16, tag=f"m{di}")
                    in0 = canvas[:, r0:r0 + HS + 1, 6:140]
                    v0 = 5 + dw
                    in1 = canvaso[:, r0 + dh:r0 + dh + HS + 1, v0:v0 + 134]
                    nc.vector.tensor_tensor(mt[:, :, 0:134], in0, in1, AOT.max)
                    mts[(dh, dw)] = mt

                # t3: 5 block-diag conv taps accumulated in PSUM
                t3p = ppool.tile([P, HS * W], fp32, tag="t3p")
                for k in range(5):
                    for half in range(2):
                        rhs = canvas[:, 1 + r0 + 4 * half:1 + r0 + 4 * half + 4,
                                     7 + k:7 + k + W]
                        nc.tensor.matmul(
                            out=t3p[:, 512 * half:512 * half + 512],
                            lhsT=wt3_s[:, k, :], rhs=rhs,
                            start=(k == 0), stop=(k == 4))

                # t7: 9 weighted tap accumulations in PSUM
                t7p = ppool.tile([P, HS * W], fp32, tag="t7p")
                for i, (_oh, _ow, d, dr, du) in enumerate(TERMS):
                    mt = mts[d]
                    for half in range(2):
                        rhs = mt[:, 4 * half + dr + 1:4 * half + dr + 1 + 4,
                                 du + 3:du + 3 + W]
                        nc.tensor.matmul(
                            out=t7p[:, 512 * half:512 * half + 512],
                            lhsT=wt7_s[:, i, :], rhs=rhs,
                            start=(i == 0), stop=(i == 8))

                # drain PSUM -> SBUF fp16 (ACT engine)
                t3s = spool.tile([P, HS, W], fp16, tag="t3s")
                t7s = spool.tile([P, HS, W], fp16, tag="t7s")
                nc.scalar.copy(out=t3s.rearrange("p a b -> p (a b)"), in_=t3p)
                nc.scalar.copy(out=t7s.rearrange("p a b -> p (a b)"), in_=t7p)

                # patch the wrap column (w=0) of t7
                nc.vector.tensor_copy(t7s[:, :, 0], t7c0[:, r0:r0 + HS])

                # final product into the output staging buffer
                if s % (OUT_ROWS // HS) == 0:
                    outs = opool.tile([P, OUT_ROWS, W], fp16, tag="outs")
                sub = s % (OUT_ROWS // HS)
                nc.vector.tensor_tensor(
                    outs[:, sub * HS:(sub + 1) * HS, :], t7s, t3s, AOT.mult)

                if sub == OUT_ROWS // HS - 1:
                    ro = (s // (OUT_ROWS // HS)) * OUT_ROWS
                    nc.gpsimd.dma_start(out=out_d.ap()[:, ro:ro + OUT_ROWS, :],
                                        in_=outs)

    nc.compile()
    return nc


_PROGRAM = None


def _get_program():
    global _PROGRAM
    if _PROGRAM is None:
        _PROGRAM = build_program()
    return _PROGRAM


def make_in_maps(inputs):
    x = np.asarray(inputs["x"], dtype=np.float32)
    wt3, wt7, w7c = build_host_weights(inputs["w3"], inputs["w7"])
    in_maps = []
    for core in range(NCORES):
        shard = x[core * NLOC:(core + 1) * NLOC].reshape(P, H, W)
        in_maps.append({"x_in": np.ascontiguousarray(shard),
                        "wt3": wt3, "wt7": wt7, "w7c": w7c})
    return in_maps


def kernel(**inputs) -> np.ndarray:
    from concourse.bass_utils import run_bass_kernel_spmd
    nc = _get_program()
    in_maps = make_in_maps(inputs)
    res = run_bass_kernel_spmd(nc, in_maps, core_ids=list(range(NCORES)))
    out = np.empty((N, C, H, W), dtype=np.float32)
    for core in range(NCORES):
        out[core * NLOC:(core + 1) * NLOC] = res.results[core]["out"].reshape(
            NLOC, C, H, W)
    return out



# revision 27
# speedup vs baseline: 1.2518x; 1.2518x over previous
"""Trainium2 Bass kernel for nn_Kernel_12281761695451725822_53472342835843.

Computation (per sample n, channel c):
  t3 = Conv1x5(x, w3)                      (channel-mixing 1x5 conv, pad 2)
  t7 = sum over 9 (oh,ow) terms of w7[c,3*ow+oh] * max(A_ohow, B_ohow)
       A = x[h, w+2*oh-2] (zero pad), B = x[h+oh-1, w+2*ow-3] (zero pad; w=0 wraps)
  out = t7 * t3

Strategy (pure data-parallel over batch: 2 samples per core, 8 cores):
  - Host pre-pads x into a zero-bordered fp16 canvas [P=128, 130, 144]
    (partitions = (n_local, c)); one dense DMA per row-chunk loads it.
  - The 9 max terms reduce to 6 shared pair-max tensors M_delta; these are
    computed per 32-row macro-strip, split across DVE and GPSIMD.
  - PE (fp16 matmuls) computes t3 (5 block-diag taps) and most of the 9
    weighted t7 taps (diagonal lhsT), accumulating in fp16 PSUM tiles.
  - Remaining t7 terms run as hybrid ACT product (per-partition scale) +
    DVE tensor-tensor add into the drained t7 tile.
  - ACT drains the PE t7 partial; DVE multiplies t7 by t3 straight out of
    fp16 PSUM; fp16 results DMA to HBM and are upcast on the host.
  - w=0 output column is recomputed exactly (roll-by-1 wrap) and patched in.
"""

import numpy as np

N, C, H, W = 16, 64, 128, 128
NCORES = 8
NLOC = N // NCORES          # samples per core
P = 128                     # partitions = NLOC * C
ROWS = H + 2                # canvas rows, storage row = h + 1
COLS = 144                  # canvas cols, storage col = u + UOFF, u in [-9, 135)
UOFF = 9
HS = 8                      # PSUM strip rows
NSTRIPS = H // HS           # 16
HSM = 32                    # macro strip rows (max computation)
NMACRO = H // HSM           # 4
MROWS = HSM + 1             # 33
MCOLS = 134                 # M tensor cols, m -> u = m - 3
HALF = 4                    # rows per matmul (PSUM bank = 512 fp32)

# term i: (oh, ow, (dh, dw) of shared max, read offset (dr, du) into M)
TERMS = [
    (0, 0, (1, 1), -1, -3),
    (0, 1, (1, -1), -1, -1),
    (0, 2, (1, -3), -1, 1),
    (1, 0, (0, 3), 0, -3),
    (1, 1, (0, 1), 0, -1),
    (1, 2, (0, 1), 0, 0),
    (2, 0, (1, -5), 0, 2),
    (2, 1, (1, -3), 0, 2),
    (2, 2, (1, -1), 0, 2),
]
DELTAS = [(1, 1), (1, -1), (1, -3), (0, 3), (0, 1), (1, -5)]
# per-delta M column range actually read by its terms (within [0, MCOLS))
DCOLS = [(0, 128), (2, 133), (4, 133), (0, 128), (2, 131), (5, 133)]

# --- engine assignment tables (tuning knobs) ---
# MAC term placement per (term, strip): True -> PE tap, False -> hybrid
# (GPSIMD cannot run tensor-tensor max, so all 6 pair-maxes live on DVE;
#  PE carries 7 of the 9 MAC taps, terms 7/8 go hybrid.)
MAC_ON_PE = [[True] * NSTRIPS for _ in range(9)]
for _t in (7, 8):
    for _s in range(NSTRIPS):
        MAC_ON_PE[_t][_s] = False
# hybrid product engine per hybrid term: "act", "dve" or "pool"
HYB_PRODUCT = {6: "act", 7: "act", 8: "act"}
# presum engine: "pool" or "dve"
PRESUM_ENG = "pool"
# final-mult engine per strip: "dve" or "pool"
MULT_ENG = ["pool"] * NSTRIPS
for _s in (0, 1, 14, 15):
    MULT_ENG[_s] = "dve"
# t3 path per strip: "drain" (ACT drain + 2x fp16 mult) or "psum" (1x mult)
T3_PATH = ["drain"] * NSTRIPS


def build_host_weights(w3, w7):
    """Host-side packing of the conv weights into PE lhsT layouts (fp16)."""
    w3 = np.asarray(w3, dtype=np.float32)
    w7 = np.asarray(w7, dtype=np.float32)
    wt3 = np.zeros((5, P, P), dtype=np.float16)
    for k in range(5):
        blk = w3[:, :, 0, k].T.astype(np.float16)  # [ci, co]
        for n in range(NLOC):
            wt3[k, n * C:(n + 1) * C, n * C:(n + 1) * C] = blk
    wt7 = np.zeros((9, P, P), dtype=np.float16)
    w7c = np.zeros((P, 9), dtype=np.float32)
    for i, (oh, ow, _d, _dr, _du) in enumerate(TERMS):
        kidx = 3 * ow + oh
        vals = np.concatenate([w7[:, kidx], w7[:, kidx]]).astype(np.float16)
        wt7[i, np.arange(P), np.arange(P)] = vals
        w7c[:, i] = vals
    return wt3, wt7, w7c


def build_program():
    """Build and compile the single-core Bass/Tile program (SPMD-replicated)."""
    import concourse.bacc as bacc
    import concourse.tile as tile
    import concourse.mybir as mybir

    fp16 = mybir.dt.float16
    fp32 = mybir.dt.float32
    AOT = mybir.AluOpType

    nc = bacc.Bacc("TRN2", target_bir_lowering=False, debug=False,
                   enable_asserts=False, num_devices=1)
    cv_in = nc.dram_tensor("cv_in", [P, ROWS, COLS], fp16, kind="ExternalInput")
    wt3_d = nc.dram_tensor("wt3", [5, P, P], fp16, kind="ExternalInput")
    wt7_d = nc.dram_tensor("wt7", [9, P, P], fp16, kind="ExternalInput")
    w7c_d = nc.dram_tensor("w7c", [P, 9], fp32, kind="ExternalInput")
    out_d = nc.dram_tensor("out", [P, H, W], fp16, kind="ExternalOutput")

    with tile.TileContext(nc) as tc:
        with (
            tc.tile_pool(name="persist", bufs=1) as persist,
            tc.tile_pool(name="mpool", bufs=2) as mpool,
            tc.tile_pool(name="ppool", bufs=2, space="PSUM") as ppool,
            tc.tile_pool(name="spool", bufs=4) as spool,
            tc.tile_pool(name="hpool", bufs=1) as hpool,
            tc.tile_pool(name="apool", bufs=2) as apool,
            tc.tile_pool(name="opool", bufs=2) as opool,
            tc.tile_pool(name="fixp", bufs=2) as fixp,
        ):
            canvas = persist.tile([P, ROWS, COLS], fp16, tag="canvas")
            wt3_s = persist.tile([P, 5, P], fp16, tag="wt3")
            wt7_s = persist.tile([P, 9, P], fp16, tag="wt7")
            w7c_s = persist.tile([P, 9], fp32, tag="w7c")
            t7c0 = persist.tile([P, H], fp16, tag="t7c0")

            # weights + host-padded canvas (row chunks); wt3 first so PE
            # warmup can start, canvas chunk 0 next so maxes start early
            CHUNKS = [(0, 18), (18, 34), (34, 66), (66, 98), (98, 130)]
            nc.sync.dma_start(out=wt3_s, in_=wt3_d.ap().rearrange("k a b -> a k b"))
            nc.sync.dma_start(out=canvas[:, 0:18, :], in_=cv_in.ap()[:, 0:18, :])
            nc.sync.dma_start(out=wt7_s, in_=wt7_d.ap().rearrange("k a b -> a k b"))
            nc.sync.dma_start(out=w7c_s, in_=w7c_d.ap())
            for r0, r1 in CHUNKS[1:]:
                nc.sync.dma_start(out=canvas[:, r0:r1, :],
                                  in_=cv_in.ap()[:, r0:r1, :])

            # --- main loop, software-pipelined at strip-pair granularity:
            # phase A(g) = maxes + fixup + hybrid products + presum for pair g
            # phase B(g) = PE taps + drain/add/patch/mult/DMA for pair g
            # emission: A(0), A(1), B(0), A(2), B(1), ... so each engine's
            # in-order queue always holds the next pair's independent work.
            def hyb_terms(g):
                # hybrid terms for pair g (must be uniform within the pair)
                assert all(MAC_ON_PE[t][2 * g] == MAC_ON_PE[t][2 * g + 1]
                           for t in range(9))
                return [t for t in range(9) if not MAC_ON_PE[t][2 * g]]

            SPM = HSM // HS          # strips per macro
            PPM = SPM // 2           # pairs per macro
            NPAIR = NSTRIPS // 2
            macro_mts = {}
            pair_padd = {}

            def emit_A_max(g):
                S, pi = divmod(g, PPM)
                r0 = S * HSM
                pr0 = 2 * HS * pi
                if pi == 0:
                    mts = {}
                    for di in range(6):
                        mt = mpool.tile([P, MROWS, MCOLS], fp16, tag=f"m{di}",
                                        name=f"mt{di}_{S}")
                        mts[DELTAS[di]] = mt
                    macro_mts[S] = mts
                mts = macro_mts[S]
                for di in range(6):
                    dh, dw = DELTAS[di]
                    c0, c1 = DCOLS[di]
                    mt = mts[DELTAS[di]]
                    in0 = canvas[:, r0 + pr0:r0 + pr0 + 2 * HS + 1,
                                 6 + c0:6 + c1]
                    in1 = canvas[:, r0 + pr0 + dh:r0 + pr0 + dh + 2 * HS + 1,
                                 6 + dw + c0:6 + dw + c1]
                    nc.vector.tensor_tensor(
                        mt[:, pr0:pr0 + 2 * HS + 1, c0:c1], in0, in1, AOT.max)

                # w=0 fixup, 64-row chunks emitted with pairs 0 and 4
                if g in (0, NPAIR // 2):
                    f0, FH = (0 if g == 0 else H // 2), H // 2
                    for i, (oh, ow, _d, _dr, _du) in enumerate(TERMS):
                        tmp = fixp.tile([P, FH], fp16, tag=f"fixtmp{i}",
                                        name=f"fix{i}_{g}")
                        a_ap = canvas[:, 1 + f0:1 + f0 + FH, 2 * oh + 7]
                        b_ap = canvas[:, f0 + oh:f0 + oh + FH, 134 + 2 * ow]
                        nc.vector.tensor_tensor(tmp, a_ap, b_ap, AOT.max)
                        nc.vector.scalar_tensor_tensor(
                            out=t7c0[:, f0:f0 + FH], in0=tmp,
                            scalar=w7c_s[:, i:i + 1],
                            in1=(tmp if i == 0 else t7c0[:, f0:f0 + FH]),
                            op0=AOT.mult,
                            op1=(AOT.bypass if i == 0 else AOT.add))

            def emit_A_prod(g):
                S, pi = divmod(g, PPM)
                pr0 = 2 * HS * pi
                mts = macro_mts[S]
                # hybrid products + pre-sum for this pair
                prods = {}
                HYB = hyb_terms(g)
                for t in HYB:
                    _oh, _ow, d, dr, du = TERMS[t]
                    mt = mts[d]
                    mread = mt[:, pr0 + dr + 1:pr0 + dr + 1 + 2 * HS,
                               du + 3:du + 3 + W]
                    pt = hpool.tile([P, 2 * HS, W], fp16, tag=f"p{t}",
                                    name=f"p{t}_{g}")
                    peng = HYB_PRODUCT[t]
                    if peng == "act":
                        nc.scalar.mul(pt, mread, w7c_s[:, t:t + 1])
                    elif peng == "pool":
                        nc.gpsimd.tensor_scalar(pt, mread, w7c_s[:, t:t + 1],
                                                None, AOT.mult)
                    else:
                        nc.vector.tensor_scalar(pt, mread, w7c_s[:, t:t + 1],
                                                None, AOT.mult)
                    prods[t] = pt
                padd = apool.tile([P, 2 * HS, W], fp16, tag="padd",
                                  name=f"padd_{g}")
                pl = [prods[t] for t in HYB]
                aeng = nc.gpsimd if PRESUM_ENG == "pool" else nc.vector
                if len(pl) == 1:
                    padd = pl[0]
                else:
                    aeng.tensor_tensor(padd, pl[0], pl[1], AOT.add)
                    for pt in pl[2:]:
                        aeng.tensor_tensor(padd, padd, pt, AOT.add)
                pair_padd[g] = padd.rearrange("p a b -> p (a b)")

            def emit_B(g):
                S, pi = divmod(g, PPM)
                mts = macro_mts[S]
                padd_flat = pair_padd.pop(g)
                subs = [2 * pi, 2 * pi + 1]
                tiles = {}
                for sub in subs:
                    s = SPM * S + sub
                    t3p = ppool.tile([P, HS * W], fp32, tag="t3p",
                                     name=f"t3p_{s}")
                    tiles[sub] = [t3p]
                    nc.tensor.matmul(out=t3p[:, 0:4], lhsT=wt3_s[:, 0, :],
                                     rhs=wt3_s[:, 0, 0:4],
                                     start=True, stop=False)
                    for k in range(5):
                        for hf in range(2):
                            rhs = canvas[:, 1 + HS * s + HALF * hf:
                                         1 + HS * s + HALF * hf + HALF,
                                         7 + k:7 + k + W]
                            nc.tensor.matmul(
                                out=t3p[:, 512 * hf:512 * hf + 512],
                                lhsT=wt3_s[:, k, :], rhs=rhs,
                                start=(k == 0), stop=(k == 4))
                for sub in subs:
                    s = SPM * S + sub
                    pe_terms = [t for t in range(9) if MAC_ON_PE[t][s]]
                    t7p = ppool.tile([P, HS * W], fp32, tag="t7p",
                                     name=f"t7p_{s}")
                    tiles[sub].append(t7p)
                    nc.tensor.matmul(out=t7p[:, 0:4], lhsT=wt3_s[:, 0, :],
                                     rhs=wt3_s[:, 0, 0:4],
                                     start=True, stop=False)
                    for j, t in enumerate(pe_terms):
                        _oh, _ow, d, dr, du = TERMS[t]
                        mt = mts[d]
                        for hf in range(2):
                            rb = HS * sub + HALF * hf + dr + 1
                            rhs = mt[:, rb:rb + HALF, du + 3:du + 3 + W]
                            nc.tensor.matmul(
                                out=t7p[:, 512 * hf:512 * hf + 512],
                                lhsT=wt7_s[:, t, :], rhs=rhs,
                                start=(j == 0),
                                stop=(j == len(pe_terms) - 1))
                for sub in subs:
                    s = SPM * S + sub
                    t3p, t7p = tiles[sub]
                    t7s = spool.tile([P, HS, W], fp16, tag="t7s",
                                     name=f"t7s_{s}")
                    t7s_flat = t7s.rearrange("p a b -> p (a b)")
                    nc.scalar.copy(out=t7s_flat, in_=t7p)
                    lo = HS * W * (sub - 2 * pi)
                    nc.vector.tensor_tensor(
                        t7s_flat, padd_flat[:, lo:lo + HS * W], t7s_flat,
                        AOT.add)
                    nc.vector.tensor_copy(t7s[:, :, 0],
                                          t7c0[:, HS * s:HS * s + HS])
                    outs = opool.tile([P, HS, W], fp16, tag="outs",
                                      name=f"outs_{s}")
                    meng = nc.gpsimd if MULT_ENG[s] == "pool" else nc.vector
                    if T3_PATH[s] == "drain":
                        t3s = spool.tile([P, HS, W], fp16, tag="t3s",
                                         name=f"t3s_{s}")
                        nc.scalar.copy(
                            out=t3s.rearrange("p a b -> p (a b)"), in_=t3p)
                        meng.tensor_tensor(
                            outs.rearrange("p a b -> p (a b)"), t7s_flat,
                            t3s.rearrange("p a b -> p (a b)"), AOT.mult)
                    else:
                        nc.vector.tensor_tensor(
                            outs.rearrange("p a b -> p (a b)"), t7s_flat, t3p,
                            AOT.mult)
                    nc.sync.dma_start(
                        out=out_d.ap()[:, HS * s:HS * s + HS, :], in_=outs)

            emit_A_max(0)
            emit_A_prod(0)
            for g in range(NPAIR):
                if g + 1 < NPAIR:
                    emit_A_max(g + 1)
                emit_B(g)
                if g + 1 < NPAIR:
                    emit_A_prod(g + 1)

    nc.compile()
    return nc


_PROGRAM = None


def _get_program():
    global _PROGRAM
    if _PROGRAM is None:
        _PROGRAM = build_program()
    return _PROGRAM


def make_in_maps(inputs):
    x = np.asarray(inputs["x"], dtype=np.float32)
    wt3, wt7, w7c = build_host_weights(inputs["w3"], inputs["w7"])
    in_maps = []
    for core in range(NCORES):
        shard = x[core * NLOC:(core + 1) * NLOC].reshape(P, H, W)
        cv = np.zeros((P, ROWS, COLS), dtype=np.float16)
        cv[:, 1:1 + H, UOFF:UOFF + W] = shard.astype(np.float16)
        in_maps.append({"cv_in": cv, "wt3": wt3, "wt7": wt7, "w7c": w7c})
    return in_maps


def kernel(**inputs) -> np.ndarray:
    from concourse.bass_utils import run_bass_kernel_spmd
    nc = _get_program()
    in_maps = make_in_maps(inputs)
    res = run_bass_kernel_spmd(nc, in_maps, core_ids=list(range(NCORES)))
    out = np.empty((N, C, H, W), dtype=np.float32)
    for core in range(NCORES):
        out[core * NLOC:(core + 1) * NLOC] = \
            res.results[core]["out"].astype(np.float32).reshape(NLOC, C, H, W)
    return out
